# revision 9
# baseline (speedup 1.0000x reference)
"""Trainium2 Bass kernel for multi-head attention decode step with KV cache.

Problem shapes (hardcoded): x[16,32,4096], wq/wk/wv/wo[4096,4096],
k_cache/v_cache[16,2048,32,128], start_pos=1024 (must be multiple of 128).

Sharding: tensor-parallel over the 32 heads -> 4 heads per core on 8 cores.
wq/wk/wv column-sharded, wo row-sharded; per-core partial outputs summed on
host.

Per-core plan:
  Phase A: QT/KT = (w.T @ x.T) in [head_dim, token] layout via fp32r matmuls
           (lhsT = w tiles, rhs = x.T tiles), V in [token, cols] layout.
           RoPE applied on PSUM eviction using a deinterleaved head_dim
           permutation (evens then odds) so the rotate is a 64-partition swap.
  Phase B (h outer, b inner): per (h, b): scoresT[s, t] chunks =
           kT_chunk.T @ qT (s on psum partitions), exp via ScalarE
           (scale=1/sqrt(128); the score distribution here cannot overflow
           fp32 exp so no max subtraction), denominator via ones-vector
           matmul, A.V accumulated over s-chunks, normalization by 1/denom
           broadcast across partitions via a K=1 outer-product matmul.
           After each head finishes: its wo pass OUT[mc] (+)= wo_h.T @ attnT_h
           runs immediately so the PE-only tail hides under cache streaming.
"""

import numpy as np

B, T, D = 16, 32, 4096
HL, HD = 4, 128          # heads per core, head dim
NTOK = B * T             # 512
NC = 8

_STATE = {}
KV_BF16 = True           # stream KV caches (and Q/K/probs) in bf16


def _build(n_cached):
    import concourse.tile as tile
    from concourse import bacc, mybir
    from contextlib import ExitStack

    f32 = mybir.dt.float32
    f32r = mybir.dt.float32r
    fkv = mybir.dt.bfloat16 if KV_BF16 else f32

    SCF = n_cached // 128          # full cached s-chunks
    NCHUNK = SCF + 1               # + new-token chunk
    COLS = NCHUNK * 32             # scoresT free width

    nc = bacc.Bacc("TRN2", target_bir_lowering=False, debug=False,
                   num_devices=NC)

    xt = nc.dram_tensor("xt", [D, NTOK], f32r, kind="ExternalInput").ap()
    wqd = nc.dram_tensor("wqd", [D, HL * HD], f32r, kind="ExternalInput").ap()
    wkd = nc.dram_tensor("wkd", [D, HL * HD], f32r, kind="ExternalInput").ap()
    wvd = nc.dram_tensor("wvd", [D, HL * HD], f32r, kind="ExternalInput").ap()
    wod = nc.dram_tensor("wod", [HL * HD, D], f32r, kind="ExternalInput").ap()
    ktc = nc.dram_tensor("ktc", [B, HL, HD, n_cached], fkv,
                         kind="ExternalInput").ap()
    vcc = nc.dram_tensor("vcc", [B, HL, 128, SCF * HD], fkv,
                         kind="ExternalInput").ap()
    cosd = nc.dram_tensor("cosd", [128, NTOK], f32, kind="ExternalInput").ap()
    sind = nc.dram_tensor("sind", [128, NTOK], f32, kind="ExternalInput").ap()
    outp = nc.dram_tensor("outp", [D, NTOK], f32, kind="ExternalOutput").ap()

    SCALE = float(1.0 / np.sqrt(np.float32(HD)))

    with tile.TileContext(nc) as tc:
        with ExitStack() as outer:
            # pools that span all phases
            qk_pool = outer.enter_context(tc.tile_pool(name="qk", bufs=1))
            v_pool = outer.enter_context(tc.tile_pool(name="vnew", bufs=1))
            att_pool = outer.enter_context(tc.tile_pool(name="attn", bufs=2))
            out_sb_pool = outer.enter_context(tc.tile_pool(name="outsb",
                                                           bufs=1))
            cs_pool = outer.enter_context(tc.tile_pool(name="cs", bufs=1))
            one_pool = outer.enter_context(tc.tile_pool(name="ones", bufs=1))

            cos_sb = cs_pool.tile([128, NTOK], f32, tag="cos")
            sin_sb = cs_pool.tile([128, NTOK], f32, tag="sin")
            nc.sync.dma_start(cos_sb[:], cosd[:])
            nc.sync.dma_start(sin_sb[:], sind[:])
            ones_col = one_pool.tile([128, 1], fkv, tag="oc")  # denom lhsT
            ones_row = one_pool.tile([1, 128], f32, tag="orow")  # outer lhsT
            nc.vector.memset(ones_col[:], 1.0)
            nc.vector.memset(ones_row[:], 1.0)

            QT = [qk_pool.tile([128, NTOK], fkv, tag=f"q{m}", name=f"QT{m}")
                  for m in range(HL)]
            KT = [qk_pool.tile([128, NTOK], fkv, tag=f"k{m}", name=f"KT{m}")
                  for m in range(HL)]
            VN = [v_pool.tile([128, HL * HD], f32, tag=f"v{m}", name=f"VN{m}")
                  for m in range(4)]
            OUT = [out_sb_pool.tile([128, NTOK], f32, tag=f"o{m}",
                                    name=f"OUT{m}") for m in range(32)]

            # ---------------- Phase A: projections + rope ----------------
            with ExitStack() as pa:
                xt_pool = pa.enter_context(tc.tile_pool(name="xt", bufs=1))
                w_pool = pa.enter_context(tc.tile_pool(name="w", bufs=2))
                pp = pa.enter_context(
                    tc.tile_pool(name="pp", bufs=2, space="PSUM"))
                rope_pool = pa.enter_context(tc.tile_pool(name="rope", bufs=2))

                XT = []
                for kc in range(4):
                    t = xt_pool.tile([128, 4096], f32r, tag=f"x{kc}",
                                     name=f"XT{kc}")
                    src = xt[kc * 1024:(kc + 1) * 1024, :].rearrange(
                        "(ks p) n -> p ks n", p=128)
                    nc.sync.dma_start(
                        t[:].rearrange("p (a b) -> p a b", a=8), src)
                    XT.append(t)

                def load_w(wap, kc):
                    t = w_pool.tile([128, 4096], f32r, tag="w", name="wtile")
                    src = wap[kc * 1024:(kc + 1) * 1024, :].rearrange(
                        "(ks p) n -> p ks n", p=128)
                    nc.sync.dma_start(
                        t[:].rearrange("p (a b) -> p a b", a=8), src)
                    return t

                def rope_evict(ps, dst):
                    tsw = rope_pool.tile([128, NTOK], f32, tag="tsw")
                    nc.vector.tensor_copy(tsw[0:64, :], ps[64:128, :])
                    nc.vector.tensor_copy(tsw[64:128, :], ps[0:64, :])
                    tco = rope_pool.tile([128, NTOK], f32, tag="tco")
                    nc.vector.tensor_mul(tco[:], ps[:], cos_sb[:])
                    tsi = rope_pool.tile([128, NTOK], f32, tag="tsi")
                    nc.vector.tensor_mul(tsi[:], tsw[:], sin_sb[:])
                    nc.vector.tensor_add(dst[:], tco[:], tsi[:])

                # QT / KT projections (orientation: lhsT = w, rhs = xT)
                for wap, dsts, do_rope in ((wqd, QT, True), (wkd, KT, True),
                                           (wvd, VN, False)):
                    ps = [pp.tile([128, 512], f32, tag=f"pp{m}", name=f"pp{m}")
                          for m in range(4)]
                    for kc in range(4):
                        wt = load_w(wap, kc)
                        for ks in range(8):
                            first = (kc == 0 and ks == 0)
                            last = (kc == 3 and ks == 7)
                            for m in range(4):
                                if do_rope:
                                    # out[hd_m, tok] += w[:, m].T @ xT
                                    nc.tensor.matmul(
                                        ps[m][:],
                                        wt[:, ks * 512 + m * 128:
                                           ks * 512 + (m + 1) * 128],
                                        XT[kc][:, ks * 512:(ks + 1) * 512],
                                        start=first, stop=last)
                                else:
                                    # V: out[tok_m, cols] += xT[:, m].T @ wv
                                    nc.tensor.matmul(
                                        ps[m][:],
                                        XT[kc][:, ks * 512 + m * 128:
                                               ks * 512 + (m + 1) * 128],
                                        wt[:, ks * 512:(ks + 1) * 512],
                                        start=first, stop=last)
                    for m in range(4):
                        if do_rope:
                            rope_evict(ps[m], dsts[m])
                        else:
                            nc.vector.tensor_copy(dsts[m][:], ps[m][:])

            # ------------- Phase B: attention + interleaved wo -----------
            with ExitStack() as pb:
                wo_pool = pb.enter_context(tc.tile_pool(name="wo", bufs=2))
                kc_pool = pb.enter_context(tc.tile_pool(name="kc", bufs=2))
                vc_pool = pb.enter_context(tc.tile_pool(name="vc", bufs=2))
                pr_pool = pb.enter_context(tc.tile_pool(name="probs", bufs=2))
                vb_pool = pb.enter_context(tc.tile_pool(name="vb", bufs=2))
                rb_pool = pb.enter_context(tc.tile_pool(name="rb", bufs=2))
                sc_ps = pb.enter_context(
                    tc.tile_pool(name="scps", bufs=2, space="PSUM"))
                av_ps = pb.enter_context(
                    tc.tile_pool(name="avps", bufs=2, space="PSUM"))
                d_ps = pb.enter_context(
                    tc.tile_pool(name="dps", bufs=1, space="PSUM"))
                ob_ps = pb.enter_context(
                    tc.tile_pool(name="obps", bufs=1, space="PSUM"))
                po_ps = pb.enter_context(
                    tc.tile_pool(name="pops", bufs=2, space="PSUM"))

                for h in range(HL):
                    wot = wo_pool.tile([128, D], f32r, tag="wo", name="wot")
                    nc.sync.dma_start(wot[:], wod[h * 128:(h + 1) * 128, :])
                    ath = att_pool.tile([128, NTOK], f32r, tag="ath",
                                        name="ath")
                    for b in range(B):
                        # paired 1MB cache DMAs (b and b+1 in one transfer)
                        if b % 2 == 0:
                            kt2 = kc_pool.tile([128, 2 * n_cached], fkv,
                                               tag="kt", name="kt2")
                            nc.sync.dma_start(
                                kt2[:].rearrange("p (b s) -> p b s", b=2),
                                ktc[b:b + 2, h].rearrange("b p s -> p b s"))
                            vt2 = vc_pool.tile([128, 2 * SCF * HD], fkv,
                                               tag="vt", name="vt2")
                            nc.sync.dma_start(
                                vt2[:].rearrange("p (b s) -> p b s", b=2),
                                vcc[b:b + 2, h].rearrange("b p s -> p b s"))
                        kt = kt2[:, (b % 2) * n_cached:
                                 (b % 2 + 1) * n_cached]
                        vt = vt2[:, (b % 2) * SCF * HD:
                                 (b % 2 + 1) * SCF * HD]
                        # new V rows for this (h, b) at partition offset 0
                        vnb = vb_pool.tile([32, HD], fkv, tag="vnb")
                        nc.vector.tensor_copy(
                            vnb[:],
                            VN[b // 4][(b % 4) * 32:(b % 4) * 32 + 32,
                                       h * 128:(h + 1) * 128])

                        qs = QT[h][:, b * 32:(b + 1) * 32]

                        sp = sc_ps.tile([128, COLS], f32, tag="sp")
                        for sc in range(SCF):
                            nc.tensor.matmul(
                                sp[:, sc * 32:(sc + 1) * 32],
                                kt[:, sc * 128:(sc + 1) * 128], qs,
                                start=True, stop=True)
                        nc.tensor.matmul(
                            sp[0:32, SCF * 32:COLS],
                            KT[h][:, b * 32:(b + 1) * 32], qs,
                            start=True, stop=True)

                        pr = pr_pool.tile([128, COLS], fkv, tag="pr")
                        nc.scalar.activation(
                            pr[:, 0:SCF * 32], sp[:, 0:SCF * 32],
                            mybir.ActivationFunctionType.Exp, scale=SCALE)
                        nc.scalar.activation(
                            pr[0:32, SCF * 32:COLS], sp[0:32, SCF * 32:COLS],
                            mybir.ActivationFunctionType.Exp, scale=SCALE)

                        # denominator: ones.T @ probs chunks -> [1, 32]
                        dp = d_ps.tile([1, 32], f32, tag="dp")
                        for sc in range(SCF):
                            nc.tensor.matmul(
                                dp[:], ones_col[:],
                                pr[:, sc * 32:(sc + 1) * 32],
                                start=(sc == 0), stop=False)
                        nc.tensor.matmul(
                            dp[:], ones_col[0:32, :],
                            pr[0:32, SCF * 32:COLS], start=False, stop=True)

                        # A.V accumulation -> [hd, t]
                        ap = av_ps.tile([128, 32], f32, tag="ap")
                        for sc in range(SCF):
                            nc.tensor.matmul(
                                ap[:], vt[:, sc * 128:(sc + 1) * 128],
                                pr[:, sc * 32:(sc + 1) * 32],
                                start=(sc == 0), stop=False)
                        nc.tensor.matmul(
                            ap[:], vnb[:],
                            pr[0:32, SCF * 32:COLS], start=False, stop=True)

                        # normalize: attnT[:, b] = av * (1/denom) bcast
                        rr = rb_pool.tile([1, 32], f32, tag="rr")
                        nc.vector.reciprocal(rr[:], dp[:])
                        rbp = ob_ps.tile([128, 32], f32, tag="rbp")
                        nc.tensor.matmul(rbp[:], ones_row[:], rr[:],
                                         start=True, stop=True)
                        rbs = rb_pool.tile([128, 32], f32, tag="rbs")
                        nc.vector.tensor_copy(rbs[:], rbp[:])
                        nc.vector.tensor_mul(
                            ath[:, b * 32:(b + 1) * 32], ap[:], rbs[:])

                    # per-h wo pass: OUT[mc] (+)= wo_h[:, mc].T @ attnT_h
                    for mc in range(32):
                        po = po_ps.tile([128, NTOK], f32, tag="po")
                        nc.tensor.matmul(
                            po[:], wot[:, mc * 128:(mc + 1) * 128],
                            ath[:], start=True, stop=True)
                        if h == 0:
                            nc.vector.tensor_copy(OUT[mc][:], po[:])
                        else:
                            nc.vector.tensor_add(OUT[mc][:], OUT[mc][:],
                                                 po[:])
                        if h == HL - 1:
                            nc.sync.dma_start(
                                outp[mc * 128:(mc + 1) * 128, :], OUT[mc][:])

    nc.compile()
    return nc


def _host_prep(x, wq, wk, wv, wo, k_cache, v_cache, n_cached):
    x = np.ascontiguousarray(np.asarray(x, dtype=np.float32))
    wq = np.asarray(wq, dtype=np.float32)
    wk = np.asarray(wk, dtype=np.float32)
    wv = np.asarray(wv, dtype=np.float32)
    wo = np.asarray(wo, dtype=np.float32)
    k_cache = np.asarray(k_cache, dtype=np.float32)
    v_cache = np.asarray(v_cache, dtype=np.float32)

    SCF = n_cached // 128
    perm = np.concatenate([np.arange(0, HD, 2), np.arange(1, HD, 2)])

    xt = np.ascontiguousarray(x.reshape(NTOK, D).T)  # [D, NTOK]

    # rope tables in deinterleaved layout
    theta = (np.float32(10000.0) **
             (np.float32(-2.0) * np.arange(0, HD, 2, dtype=np.float32)
              / np.float32(HD)))                      # [64]
    freqs = np.arange(T, dtype=np.float32)[:, None] * theta[None, :]  # [T,64]
    cos_t = np.cos(freqs).astype(np.float32).T        # [64, T]
    sin_t = np.sin(freqs).astype(np.float32).T
    cos_rep = np.tile(cos_t, (1, B))                  # [64, NTOK]
    sin_rep = np.tile(sin_t, (1, B))
    cosd = np.ascontiguousarray(np.concatenate([cos_rep, cos_rep], axis=0))
    sind = np.ascontiguousarray(np.concatenate([-sin_rep, sin_rep], axis=0))

    in_maps = []
    for c in range(NC):
        hs = np.arange(c * HL, (c + 1) * HL)
        cols = (hs[:, None] * HD + perm[None, :]).reshape(-1)   # permuted q/k
        colsv = (hs[:, None] * HD + np.arange(HD)[None, :]).reshape(-1)
        wq_c = np.ascontiguousarray(wq[:, cols])
        wk_c = np.ascontiguousarray(wk[:, cols])
        wv_c = np.ascontiguousarray(wv[:, colsv])
        wo_c = np.ascontiguousarray(wo[colsv, :])
        # k cache: [b, h, hd(perm), s]
        kc_c = np.ascontiguousarray(
            k_cache[:, :n_cached][:, :, hs][:, :, :, perm]
            .transpose(0, 2, 3, 1))
        # v cache: [b, h, sp, sc, hd] -> flat [b, h, 128, SCF*HD]
        vc_c = np.ascontiguousarray(
            v_cache[:, :n_cached][:, :, hs]
            .reshape(B, SCF, 128, HL, HD)
            .transpose(0, 3, 2, 1, 4)
            .reshape(B, HL, 128, SCF * HD))
        if KV_BF16:
            import ml_dtypes
            kc_c = kc_c.astype(ml_dtypes.bfloat16)
            vc_c = vc_c.astype(ml_dtypes.bfloat16)
        in_maps.append({
            "xt": xt, "wqd": wq_c, "wkd": wk_c, "wvd": wv_c, "wod": wo_c,
            "ktc": kc_c, "vcc": vc_c, "cosd": cosd, "sind": sind,
        })
    return in_maps


def kernel(x, wq, wk, wv, wo, k_cache, v_cache, start_pos):
    from concourse import bass_utils

    n_cached = int(start_pos)
    assert n_cached % 128 == 0, "kernel assumes start_pos multiple of 128"

    if _STATE.get("n_cached") != n_cached:
        _STATE["nc"] = _build(n_cached)
        _STATE["n_cached"] = n_cached
    ncb = _STATE["nc"]

    in_maps = _host_prep(x, wq, wk, wv, wo, k_cache, v_cache, n_cached)
    res = bass_utils.run_bass_kernel_spmd(ncb, in_maps,
                                          core_ids=list(range(NC)))
    out = np.zeros((D, NTOK), dtype=np.float32)
    for c in range(NC):
        out += res.results[c]["outp"]
    return np.ascontiguousarray(out.T).reshape(B, T, D)


# revision 10
# speedup vs baseline: 212097.4519x; 212097.4519x over previous
"""Trainium2 Bass kernel for multi-head attention decode step with KV cache.

Problem shapes (hardcoded): x[16,32,4096], wq/wk/wv/wo[4096,4096],
k_cache/v_cache[16,2048,32,128], start_pos=1024 (must be multiple of 128).

Sharding: tensor-parallel over the 32 heads -> 4 heads per core on 8 cores.
wq/wk/wv column-sharded, wo row-sharded; per-core partial outputs summed on
host.

Per-core plan:
  Phase A: QT/KT = (w.T @ x.T) in [head_dim, token] layout via fp32r matmuls
           (lhsT = w tiles, rhs = x.T tiles), V in [token, cols] layout.
           RoPE applied on PSUM eviction using a deinterleaved head_dim
           permutation (evens then odds) so the rotate is a 64-partition swap.
  Phase B (h outer, b inner): per (h, b): scoresT[s, t] chunks =
           kT_chunk.T @ qT (s on psum partitions), exp via ScalarE
           (scale=1/sqrt(128); the score distribution here cannot overflow
           fp32 exp so no max subtraction), denominator via ones-vector
           matmul, A.V accumulated over s-chunks, normalization by 1/denom
           broadcast across partitions via a K=1 outer-product matmul.
           After each head finishes: its wo pass OUT[mc] (+)= wo_h.T @ attnT_h
           runs immediately so the PE-only tail hides under cache streaming.
"""

import numpy as np

B, T, D = 16, 32, 4096
HL, HD = 4, 128          # heads per core, head dim
NTOK = B * T             # 512
NC = 8

_STATE = {}
KV_BF16 = True           # stream KV caches (and Q/K/probs) in bf16


def _build(n_cached):
    import concourse.tile as tile
    from concourse import bacc, mybir
    from contextlib import ExitStack

    f32 = mybir.dt.float32
    f32r = mybir.dt.float32r
    fkv = mybir.dt.bfloat16 if KV_BF16 else f32

    SCF = n_cached // 128          # full cached s-chunks
    NCHUNK = SCF + 1               # + new-token chunk
    COLS = NCHUNK * 32             # scoresT free width

    nc = bacc.Bacc("TRN2", target_bir_lowering=False, debug=False,
                   num_devices=NC)

    xt = nc.dram_tensor("xt", [D, NTOK], f32r, kind="ExternalInput").ap()
    wqd = nc.dram_tensor("wqd", [D, HL * HD], f32r, kind="ExternalInput").ap()
    wkd = nc.dram_tensor("wkd", [D, HL * HD], f32r, kind="ExternalInput").ap()
    wvd = nc.dram_tensor("wvd", [D, HL * HD], f32r, kind="ExternalInput").ap()
    wod = nc.dram_tensor("wod", [HL * HD, D], f32r, kind="ExternalInput").ap()
    ktc = nc.dram_tensor("ktc", [B, HL, HD, n_cached], fkv,
                         kind="ExternalInput").ap()
    vcc = nc.dram_tensor("vcc", [B, HL, 128, SCF * HD], fkv,
                         kind="ExternalInput").ap()
    cosd = nc.dram_tensor("cosd", [128, NTOK], f32, kind="ExternalInput").ap()
    sind = nc.dram_tensor("sind", [128, NTOK], f32, kind="ExternalInput").ap()
    outp = nc.dram_tensor("outp", [D, NTOK], f32, kind="ExternalOutput").ap()

    SCALE = float(1.0 / np.sqrt(np.float32(HD)))

    with tile.TileContext(nc) as tc:
        with ExitStack() as outer:
            # pools that span all phases
            qk_pool = outer.enter_context(tc.tile_pool(name="qk", bufs=1))
            v_pool = outer.enter_context(tc.tile_pool(name="vnew", bufs=1))
            att_pool = outer.enter_context(tc.tile_pool(name="attn", bufs=2))
            out_sb_pool = outer.enter_context(tc.tile_pool(name="outsb",
                                                           bufs=1))
            cs_pool = outer.enter_context(tc.tile_pool(name="cs", bufs=1))
            one_pool = outer.enter_context(tc.tile_pool(name="ones", bufs=1))

            cos_sb = cs_pool.tile([128, NTOK], f32, tag="cos")
            sin_sb = cs_pool.tile([128, NTOK], f32, tag="sin")
            nc.sync.dma_start(cos_sb[:], cosd[:])
            nc.sync.dma_start(sin_sb[:], sind[:])
            ones_col = one_pool.tile([128, 1], fkv, tag="oc")  # denom lhsT
            nc.vector.memset(ones_col[:], 1.0)

            QT = [qk_pool.tile([128, NTOK], fkv, tag=f"q{m}", name=f"QT{m}")
                  for m in range(HL)]
            KT = [qk_pool.tile([128, NTOK], fkv, tag=f"k{m}", name=f"KT{m}")
                  for m in range(HL)]
            VN = [v_pool.tile([128, HL * HD], f32, tag=f"v{m}", name=f"VN{m}")
                  for m in range(4)]
            OUT = [out_sb_pool.tile([128, NTOK], f32, tag=f"o{m}",
                                    name=f"OUT{m}") for m in range(32)]

            # ---------------- Phase A: projections + rope ----------------
            with ExitStack() as pa:
                xt_pool = pa.enter_context(tc.tile_pool(name="xt", bufs=1))
                w_pool = pa.enter_context(tc.tile_pool(name="w", bufs=2))
                pp = pa.enter_context(
                    tc.tile_pool(name="pp", bufs=2, space="PSUM"))
                rope_pool = pa.enter_context(tc.tile_pool(name="rope", bufs=2))

                XT = []
                for kc in range(4):
                    t = xt_pool.tile([128, 4096], f32r, tag=f"x{kc}",
                                     name=f"XT{kc}")
                    src = xt[kc * 1024:(kc + 1) * 1024, :].rearrange(
                        "(ks p) n -> p ks n", p=128)
                    nc.sync.dma_start(
                        t[:].rearrange("p (a b) -> p a b", a=8), src)
                    XT.append(t)

                def load_w(wap, kc):
                    t = w_pool.tile([128, 4096], f32r, tag="w", name="wtile")
                    src = wap[kc * 1024:(kc + 1) * 1024, :].rearrange(
                        "(ks p) n -> p ks n", p=128)
                    nc.sync.dma_start(
                        t[:].rearrange("p (a b) -> p a b", a=8), src)
                    return t

                def rope_evict(ps, dst):
                    tsw = rope_pool.tile([128, NTOK], f32, tag="tsw")
                    nc.vector.tensor_copy(tsw[0:64, :], ps[64:128, :])
                    nc.vector.tensor_copy(tsw[64:128, :], ps[0:64, :])
                    tco = rope_pool.tile([128, NTOK], f32, tag="tco")
                    nc.vector.tensor_mul(tco[:], ps[:], cos_sb[:])
                    tsi = rope_pool.tile([128, NTOK], f32, tag="tsi")
                    nc.vector.tensor_mul(tsi[:], tsw[:], sin_sb[:])
                    nc.vector.tensor_add(dst[:], tco[:], tsi[:])

                # QT / KT projections (orientation: lhsT = w, rhs = xT)
                for wap, dsts, do_rope in ((wqd, QT, True), (wkd, KT, True),
                                           (wvd, VN, False)):
                    ps = [pp.tile([128, 512], f32, tag=f"pp{m}", name=f"pp{m}")
                          for m in range(4)]
                    for kc in range(4):
                        wt = load_w(wap, kc)
                        for ks in range(8):
                            first = (kc == 0 and ks == 0)
                            last = (kc == 3 and ks == 7)
                            for m in range(4):
                                if do_rope:
                                    # out[hd_m, tok] += w[:, m].T @ xT
                                    nc.tensor.matmul(
                                        ps[m][:],
                                        wt[:, ks * 512 + m * 128:
                                           ks * 512 + (m + 1) * 128],
                                        XT[kc][:, ks * 512:(ks + 1) * 512],
                                        start=first, stop=last)
                                else:
                                    # V: out[tok_m, cols] += xT[:, m].T @ wv
                                    nc.tensor.matmul(
                                        ps[m][:],
                                        XT[kc][:, ks * 512 + m * 128:
                                               ks * 512 + (m + 1) * 128],
                                        wt[:, ks * 512:(ks + 1) * 512],
                                        start=first, stop=last)
                    for m in range(4):
                        if do_rope:
                            rope_evict(ps[m], dsts[m])
                        else:
                            nc.vector.tensor_copy(dsts[m][:], ps[m][:])

            # ------------- Phase B: attention + interleaved wo -----------
            with ExitStack() as pb:
                wo_pool = pb.enter_context(tc.tile_pool(name="wo", bufs=2))
                kc_pool = pb.enter_context(tc.tile_pool(name="kc", bufs=3))
                vc_pool = pb.enter_context(tc.tile_pool(name="vc", bufs=3))
                pr_pool = pb.enter_context(tc.tile_pool(name="probs", bufs=2))
                vb_pool = pb.enter_context(tc.tile_pool(name="vb", bufs=2))
                rb_pool = pb.enter_context(tc.tile_pool(name="rb", bufs=2))
                sc_ps = pb.enter_context(
                    tc.tile_pool(name="scps", bufs=2, space="PSUM"))
                av_ps = pb.enter_context(
                    tc.tile_pool(name="avps", bufs=2, space="PSUM"))
                d_ps = pb.enter_context(
                    tc.tile_pool(name="dps", bufs=2, space="PSUM"))
                po_ps = pb.enter_context(
                    tc.tile_pool(name="pops", bufs=2, space="PSUM"))

                for h in range(HL):
                    wot = wo_pool.tile([128, D], f32r, tag="wo", name="wot")
                    nc.sync.dma_start(wot[:], wod[h * 128:(h + 1) * 128, :])
                    ath = att_pool.tile([128, NTOK], f32r, tag="ath",
                                        name="ath")
                    for b in range(B):
                        # paired 1MB cache DMAs (b and b+1 in one transfer)
                        if b % 2 == 0:
                            kt2 = kc_pool.tile([128, 2 * n_cached], fkv,
                                               tag="kt", name="kt2")
                            nc.sync.dma_start(
                                kt2[:].rearrange("p (b s) -> p b s", b=2),
                                ktc[b:b + 2, h].rearrange("b p s -> p b s"))
                            vt2 = vc_pool.tile([128, 2 * SCF * HD], fkv,
                                               tag="vt", name="vt2")
                            nc.sync.dma_start(
                                vt2[:].rearrange("p (b s) -> p b s", b=2),
                                vcc[b:b + 2, h].rearrange("b p s -> p b s"))
                        kt = kt2[:, (b % 2) * n_cached:
                                 (b % 2 + 1) * n_cached]
                        vt = vt2[:, (b % 2) * SCF * HD:
                                 (b % 2 + 1) * SCF * HD]
                        # new V rows for this (h, b) at partition offset 0
                        vnb = vb_pool.tile([32, HD], fkv, tag="vnb")
                        nc.vector.tensor_copy(
                            vnb[:],
                            VN[b // 4][(b % 4) * 32:(b % 4) * 32 + 32,
                                       h * 128:(h + 1) * 128])

                        qs = QT[h][:, b * 32:(b + 1) * 32]

                        sp = sc_ps.tile([128, COLS], f32, tag="sp")
                        for sc in range(SCF):
                            nc.tensor.matmul(
                                sp[:, sc * 32:(sc + 1) * 32],
                                kt[:, sc * 128:(sc + 1) * 128], qs,
                                start=True, stop=True)
                        nc.tensor.matmul(
                            sp[0:32, SCF * 32:COLS],
                            KT[h][:, b * 32:(b + 1) * 32], qs,
                            start=True, stop=True)

                        pr = pr_pool.tile([128, COLS], fkv, tag="pr")
                        nc.scalar.activation(
                            pr[:, 0:SCF * 32], sp[:, 0:SCF * 32],
                            mybir.ActivationFunctionType.Exp, scale=SCALE)
                        nc.scalar.activation(
                            pr[0:32, SCF * 32:COLS], sp[0:32, SCF * 32:COLS],
                            mybir.ActivationFunctionType.Exp, scale=SCALE)

                        # denominator: ones.T @ probs chunks -> [1, 32]
                        dp = d_ps.tile([1, 32], f32, tag="dp")
                        for sc in range(SCF):
                            nc.tensor.matmul(
                                dp[:], ones_col[:],
                                pr[:, sc * 32:(sc + 1) * 32],
                                start=(sc == 0), stop=False)
                        nc.tensor.matmul(
                            dp[:], ones_col[0:32, :],
                            pr[0:32, SCF * 32:COLS], start=False, stop=True)

                        # A.V accumulation -> [hd, t]
                        ap = av_ps.tile([128, 32], f32, tag="ap")
                        for sc in range(SCF):
                            nc.tensor.matmul(
                                ap[:], vt[:, sc * 128:(sc + 1) * 128],
                                pr[:, sc * 32:(sc + 1) * 32],
                                start=(sc == 0), stop=False)
                        nc.tensor.matmul(
                            ap[:], vnb[:],
                            pr[0:32, SCF * 32:COLS], start=False, stop=True)

                        # normalize: attnT[:, b] = av * (1/denom) bcast
                        rr = rb_pool.tile([1, 32], f32, tag="rr")
                        nc.vector.reciprocal(rr[:], dp[:])
                        rbs = rb_pool.tile([128, 32], f32, tag="rbs")
                        nc.gpsimd.partition_broadcast(rbs[:], rr[:])
                        nc.vector.tensor_mul(
                            ath[:, b * 32:(b + 1) * 32], ap[:], rbs[:])

                        # half-pass of wo as soon as tokens 0:256 / 256:512
                        # of this head's attnT are complete
                        if b % 8 == 7:
                            hf = b // 8
                            lo, hi = hf * 256, (hf + 1) * 256
                            for mc in range(32):
                                po = po_ps.tile([128, 256], f32, tag="po")
                                nc.tensor.matmul(
                                    po[:], wot[:, mc * 128:(mc + 1) * 128],
                                    ath[:, lo:hi], start=True, stop=True)
                                if h == 0:
                                    nc.vector.tensor_copy(
                                        OUT[mc][:, lo:hi], po[:])
                                else:
                                    nc.vector.tensor_add(
                                        OUT[mc][:, lo:hi],
                                        OUT[mc][:, lo:hi], po[:])
                                if h == HL - 1:
                                    nc.sync.dma_start(
                                        outp[mc * 128:(mc + 1) * 128, lo:hi],
                                        OUT[mc][:, lo:hi])

    nc.compile()
    return nc


def _host_prep(x, wq, wk, wv, wo, k_cache, v_cache, n_cached):
    x = np.ascontiguousarray(np.asarray(x, dtype=np.float32))
    wq = np.asarray(wq, dtype=np.float32)
    wk = np.asarray(wk, dtype=np.float32)
    wv = np.asarray(wv, dtype=np.float32)
    wo = np.asarray(wo, dtype=np.float32)
    k_cache = np.asarray(k_cache, dtype=np.float32)
    v_cache = np.asarray(v_cache, dtype=np.float32)

    SCF = n_cached // 128
    perm = np.concatenate([np.arange(0, HD, 2), np.arange(1, HD, 2)])

    xt = np.ascontiguousarray(x.reshape(NTOK, D).T)  # [D, NTOK]

    # rope tables in deinterleaved layout
    theta = (np.float32(10000.0) **
             (np.float32(-2.0) * np.arange(0, HD, 2, dtype=np.float32)
              / np.float32(HD)))                      # [64]
    freqs = np.arange(T, dtype=np.float32)[:, None] * theta[None, :]  # [T,64]
    cos_t = np.cos(freqs).astype(np.float32).T        # [64, T]
    sin_t = np.sin(freqs).astype(np.float32).T
    cos_rep = np.tile(cos_t, (1, B))                  # [64, NTOK]
    sin_rep = np.tile(sin_t, (1, B))
    cosd = np.ascontiguousarray(np.concatenate([cos_rep, cos_rep], axis=0))
    sind = np.ascontiguousarray(np.concatenate([-sin_rep, sin_rep], axis=0))

    in_maps = []
    for c in range(NC):
        hs = np.arange(c * HL, (c + 1) * HL)
        cols = (hs[:, None] * HD + perm[None, :]).reshape(-1)   # permuted q/k
        colsv = (hs[:, None] * HD + np.arange(HD)[None, :]).reshape(-1)
        wq_c = np.ascontiguousarray(wq[:, cols])
        wk_c = np.ascontiguousarray(wk[:, cols])
        wv_c = np.ascontiguousarray(wv[:, colsv])
        wo_c = np.ascontiguousarray(wo[colsv, :])
        # k cache: [b, h, hd(perm), s]
        kc_c = np.ascontiguousarray(
            k_cache[:, :n_cached][:, :, hs][:, :, :, perm]
            .transpose(0, 2, 3, 1))
        # v cache: [b, h, sp, sc, hd] -> flat [b, h, 128, SCF*HD]
        vc_c = np.ascontiguousarray(
            v_cache[:, :n_cached][:, :, hs]
            .reshape(B, SCF, 128, HL, HD)
            .transpose(0, 3, 2, 1, 4)
            .reshape(B, HL, 128, SCF * HD))
        if KV_BF16:
            import ml_dtypes
            kc_c = kc_c.astype(ml_dtypes.bfloat16)
            vc_c = vc_c.astype(ml_dtypes.bfloat16)
        in_maps.append({
            "xt": xt, "wqd": wq_c, "wkd": wk_c, "wvd": wv_c, "wod": wo_c,
            "ktc": kc_c, "vcc": vc_c, "cosd": cosd, "sind": sind,
        })
    return in_maps


def kernel(x, wq, wk, wv, wo, k_cache, v_cache, start_pos):
    from concourse import bass_utils

    n_cached = int(start_pos)
    assert n_cached % 128 == 0, "kernel assumes start_pos multiple of 128"

    if _STATE.get("n_cached") != n_cached:
        _STATE["nc"] = _build(n_cached)
        _STATE["n_cached"] = n_cached
    ncb = _STATE["nc"]

    in_maps = _host_prep(x, wq, wk, wv, wo, k_cache, v_cache, n_cached)
    res = bass_utils.run_bass_kernel_spmd(ncb, in_maps,
                                          core_ids=list(range(NC)))
    out = np.zeros((D, NTOK), dtype=np.float32)
    for c in range(NC):
        out += res.results[c]["outp"]
    return np.ascontiguousarray(out.T).reshape(B, T, D)
